# revision 23
# baseline (speedup 1.0000x reference)
"""Dense correspondence contrastive loss kernel for Trainium2 (8 NeuronCores).

Problem (B=32, C=64, N=1024 spatial positions per sample):
  - l2-normalize q_b/k_b/q_grid/k_grid along C
  - sim[b,i,j] = <qb_hat[b,:,i], kb_hat[b,:,j]>; idx = argmax_j sim
  - pos[b,i] = <qg_hat[b,:,i], kg_hat[b,:,idx[b,i]]> / 0.1
  - neg[b,i] = <qg_hat[b,:,i], kg_hat[neg_idx[b],:,i]> / 0.1
    (neg_idx from labels/neg_noise -- O(B^2) host-side index prep)
  - loss = mean(log(exp(pos)+exp(neg)+1e-6) - pos)

Sharding: data-parallel over batch, 4 samples per core.  Per core the
device does: k_b column norms (PE colsum into a partition-parallel
[8,128] layout + K=1 broadcast matmuls), bf16 sim matmuls into fp32
PSUM, argmax via DVE max-reduce + fused (sim>=max)*iota row-sum
(exact: no fp32 ties), indirect-DMA gather of the matched k_grid rows
(host pre-transposes q_grid/k_grid to [N, C] so rows are contiguous
and channel norms reduce along the free dim), then a batched loss
tail.  Host sums 8 partial scalars.

Key algebraic shortcut: q_b normalization is skipped entirely --
argmax_j over j is invariant to the per-row positive scale 1/|q_b[:,i]|.
bf16 is used only for the sim matmul operands (PSUM accumulates fp32);
measured end-to-end impact ~4e-4 relative, from ~140/32768 argmax
flips between near-equal similarities.
"""

import os
import numpy as np

B = 32
C = 64
N = 1024
NCORES = 8
SPC = B // NCORES          # samples per core
MT = N // 128              # 128-row m-tiles per sample
NT = SPC * MT              # accumulator columns per core
TEMP = 0.1
EPS_LOSS = 1e-6

LAST_EXEC_TIME_NS = None
_CACHE = {}


def _build_module():
    import concourse.bass as bass
    import concourse.bacc as bacc
    import concourse.tile as tile
    from concourse import mybir
    from contextlib import ExitStack

    F32 = mybir.dt.float32
    BF16 = mybir.dt.bfloat16
    U32 = mybir.dt.uint32
    AX = mybir.AxisListType
    ALU = mybir.AluOpType
    ACTF = mybir.ActivationFunctionType

    nc = bacc.Bacc("TRN2", target_bir_lowering=False, debug=False,
                   num_devices=NCORES)

    qb_d = nc.dram_tensor("qb", [SPC * C, N], F32, kind="ExternalInput")
    kb_d = nc.dram_tensor("kb", [SPC * C, N], F32, kind="ExternalInput")
    qgt_d = nc.dram_tensor("qgt", [SPC * N, C], F32, kind="ExternalInput")
    kgt_d = nc.dram_tensor("kgt", [SPC * N, C], F32, kind="ExternalInput")
    kngt_d = nc.dram_tensor("kngt", [SPC * N, C], F32, kind="ExternalInput")
    ind_d = nc.dram_tensor("cst_ind", [C, MT * MT], F32, kind="ExternalInput")
    indT_d = nc.dram_tensor("cst_indT", [MT, MT * C], F32, kind="ExternalInput")
    out_d = nc.dram_tensor("out", [1, 1], F32, kind="ExternalOutput")

    with tile.TileContext(nc) as tc, ExitStack() as ctx:
        const = ctx.enter_context(tc.tile_pool(name="const", bufs=1))
        accum = ctx.enter_context(tc.tile_pool(name="accum", bufs=1))
        io = ctx.enter_context(tc.tile_pool(name="io", bufs=2))
        mt_p = ctx.enter_context(tc.tile_pool(name="mt", bufs=4))
        qg_p = ctx.enter_context(tc.tile_pool(name="qg", bufs=2))
        scr = ctx.enter_context(tc.tile_pool(name="scr", bufs=4))
        ps_sim = ctx.enter_context(tc.tile_pool(name="ps_sim", bufs=3, space="PSUM"))
        ps_aux = ctx.enter_context(tc.tile_pool(name="ps_aux", bufs=2, space="PSUM"))

        iota = const.tile([128, N], F32)
        nc.gpsimd.iota(iota[:], pattern=[[1, N]], base=0, channel_multiplier=0,
                       allow_small_or_imprecise_dtypes=True)
        ones64 = const.tile([C, 1], F32)
        nc.vector.memset(ones64[:], 1.0)
        ones1x64 = const.tile([1, C], F32)
        nc.vector.memset(ones1x64[:], 1.0)
        ones128 = const.tile([128, 1], F32)
        nc.vector.memset(ones128[:], 1.0)
        b24 = const.tile([128, 1], F32)
        nc.vector.memset(b24[:], 1e-24)
        # indicator weights: column j ones -> colsum of chunk j accumulates
        # into PSUM partition j (PE output base partition must be 0)
        # indicator weight matrices (host-supplied constants):
        # inds[j] [C,MT]: column j ones -> colsum of chunk j lands in PSUM
        # partition j; indTs[j] [MT,C]: row j ones -> broadcasts chunk j of
        # the [MT,128] reciprocal tile over all C output partitions
        ind_sb = const.tile([C, MT * MT], F32)
        nc.sync.dma_start(ind_sb[:], ind_d[:, :])
        indT_sb = const.tile([MT, MT * C], F32)
        nc.sync.dma_start(indT_sb[:], indT_d[:, :])
        inds = [ind_sb[:, j * MT:(j + 1) * MT] for j in range(MT)]
        indTs = [indT_sb[:, j * C:(j + 1) * C] for j in range(MT)]

        # merged norm accumulators: cols [0:NT)=qg, [NT:2NT)=k_gathered,
        # [2NT:3NT)=k_neg; one sqrt+reciprocal in the tail covers all three
        ssqa = accum.tile([128, 3 * NT], F32, tag="ssqa")
        dps = accum.tile([128, NT], F32, tag="dps")
        dns = accum.tile([128, NT], F32, tag="dns")

        for b in range(SPC):
            kb_t = io.tile([C, N], F32, tag="kb")
            nc.sync.dma_start(kb_t[:], kb_d[b * C:(b + 1) * C, :])
            qb_t = io.tile([C, N], F32, tag="qb")
            nc.sync.dma_start(qb_t[:], qb_d[b * C:(b + 1) * C, :])

            # k_b column norms: colsums land partition-parallel ([8,128], one
            # 128-col chunk per partition) so the reciprocal runs at 128 free
            # elems instead of 1024
            sq = io.tile([C, N], F32, tag="sq")
            nc.scalar.activation(sq[:], kb_t[:], ACTF.Square)
            ssq_ps = ps_aux.tile([MT, 128], F32, tag="aux")
            for j in range(MT):
                nc.tensor.matmul(ssq_ps[:], inds[j],
                                 sq[:, j * 128:(j + 1) * 128],
                                 start=(j == 0), stop=(j == MT - 1))
            rn_s = io.tile([MT, 128], F32, tag="rn_s")
            nc.scalar.activation(rn_s[:], ssq_ps[:], ACTF.Sqrt, bias=b24[0:MT, :])
            rn8 = io.tile([MT, 128], F32, tag="rn8")
            nc.vector.reciprocal(rn8[:], rn_s[:])

            # broadcast 1/|k_j| over C (K=1 matmul per 128-col chunk) and
            # scale k_b, emitting bf16 for the sim matmul
            kbh = io.tile([C, N], BF16, tag="kbh")
            for h in range(2):
                rnb_ps = ps_aux.tile([C, 512], F32, tag="aux")
                for j in range(4):
                    nc.tensor.matmul(rnb_ps[:, j * 128:(j + 1) * 128],
                                     indTs[4 * h + j], rn8[:],
                                     start=True, stop=True)
                nc.vector.tensor_mul(kbh[:, h * 512:(h + 1) * 512],
                                     kb_t[:, h * 512:(h + 1) * 512], rnb_ps[:])
            qb_bf = io.tile([C, N], BF16, tag="qb_bf")
            nc.scalar.activation(qb_bf[:], qb_t[:], ACTF.Copy)

            # whole-sample strided loads: [128, MT*C] with m-tile m in columns
            # [m*C, (m+1)*C); issued on the ACT HWDGE ring to unload Sync-seq
            qgs = qg_p.tile([128, MT * C], F32, tag="qg")
            nc.scalar.dma_start(
                qgs[:], qgt_d[b * N:(b + 1) * N, :].rearrange("(m p) c -> p m c", p=128))
            kngs = qg_p.tile([128, MT * C], F32, tag="kng")
            nc.scalar.dma_start(
                kngs[:], kngt_d[b * N:(b + 1) * N, :].rearrange("(m p) c -> p m c", p=128))

            # channel sum-squares, batched: one ACT square + one 3D reduce
            sqg = scr.tile([128, MT * C], F32, tag="sq512")
            nc.scalar.activation(sqg[:], qgs[:], ACTF.Square)
            nc.vector.tensor_reduce(ssqa[:, b * MT:(b + 1) * MT],
                                    sqg[:].rearrange("p (m c) -> p m c", c=C),
                                    axis=AX.X, op=ALU.add)
            sqn = scr.tile([128, MT * C], F32, tag="sq512")
            nc.scalar.activation(sqn[:], kngs[:], ACTF.Square)
            nc.vector.tensor_reduce(ssqa[:, 2 * NT + b * MT:2 * NT + (b + 1) * MT],
                                    sqn[:].rearrange("p (m c) -> p m c", c=C),
                                    axis=AX.X, op=ALU.add)
            # raw negative dots: product on gpsimd, chunk-reduce on DVE
            prodn = scr.tile([128, MT * C], F32, tag="sq512")
            nc.gpsimd.tensor_mul(prodn[:], qgs[:], kngs[:])
            nc.vector.tensor_reduce(dns[:, b * MT:(b + 1) * MT],
                                    prodn[:].rearrange("p (m c) -> p m c", c=C),
                                    axis=AX.X, op=ALU.add)

            idxs = mt_p.tile([128, MT], F32, tag="idxs")
            for m in range(MT):
                sim_ps = ps_sim.tile([128, N], F32, tag="sim")
                nc.tensor.matmul(sim_ps[:, 0:512], qb_bf[:, m * 128:(m + 1) * 128],
                                 kbh[:, 0:512], start=True, stop=True)
                nc.tensor.matmul(sim_ps[:, 512:N], qb_bf[:, m * 128:(m + 1) * 128],
                                 kbh[:, 512:N], start=True, stop=True)
                gmax = mt_p.tile([128, 1], F32, tag="gmax")
                nc.vector.reduce_max(gmax[:], sim_ps[:], axis=AX.X)
                # evict to SBUF so the locate pass runs in the 2x DVE mode
                # (DMA cannot read PSUM; ACT does the copy, overlapped with MAX)
                sim_sb = scr.tile([128, N], F32, tag="sims")
                nc.scalar.activation(sim_sb[:], sim_ps[:], ACTF.Copy)
                big = scr.tile([128, N], F32, tag="big")
                nc.vector.scalar_tensor_tensor(
                    big[:], sim_sb[:], gmax[:], iota[:],
                    op0=ALU.is_ge, op1=ALU.mult, accum_out=idxs[:, m:m + 1])

            # argmax columns -> clamped u32 row indices into the flat [SPC*N, C]
            # transposed k_grid (tie-sum clamp is belt-and-braces; fp32 sims tie
            # with probability ~0)
            idxc = mt_p.tile([128, MT], F32, tag="idxc")
            nc.vector.tensor_scalar(idxc[:], idxs[:], 1023.0, float(b * N),
                                    op0=ALU.min, op1=ALU.add)
            idxu = mt_p.tile([128, MT], U32, tag="idxu")
            nc.vector.tensor_copy(idxu[:], idxc[:])

            import concourse.bass as bass_mod
            kgas = qg_p.tile([128, MT * C], F32, tag="kga")
            for m in range(MT):
                nc.gpsimd.indirect_dma_start(
                    kgas[:, m * C:(m + 1) * C], None, kgt_d.ap(),
                    bass_mod.IndirectOffsetOnAxis(ap=idxu[:, m:m + 1], axis=0))
            sqk = scr.tile([128, MT * C], F32, tag="sq512")
            nc.scalar.activation(sqk[:], kgas[:], ACTF.Square)
            nc.vector.tensor_reduce(ssqa[:, NT + b * MT:NT + (b + 1) * MT],
                                    sqk[:].rearrange("p (m c) -> p m c", c=C),
                                    axis=AX.X, op=ALU.add)
            prodp = scr.tile([128, MT * C], F32, tag="sq512")
            nc.gpsimd.tensor_mul(prodp[:], qgs[:], kgas[:])
            nc.vector.tensor_reduce(dps[:, b * MT:(b + 1) * MT],
                                    prodp[:].rearrange("p (m c) -> p m c", c=C),
                                    axis=AX.X, op=ALU.add)

        # batched loss tail over the [128, NT] accumulators; the 1/TEMP=10
        # factor rides along as the stt immediate
        ra_s = accum.tile([128, 3 * NT], F32, tag="ra_s")
        nc.scalar.activation(ra_s[:], ssqa[:], ACTF.Sqrt, bias=b24[:])
        ra = accum.tile([128, 3 * NT], F32, tag="ra")
        nc.vector.reciprocal(ra[:], ra_s[:])

        t1 = accum.tile([128, NT], F32, tag="t1")
        nc.vector.tensor_mul(t1[:], dps[:], ra[:, 0:NT])
        pos = accum.tile([128, NT], F32, tag="pos")
        nc.vector.scalar_tensor_tensor(pos[:], t1[:], 10.0, ra[:, NT:2 * NT],
                                       op0=ALU.mult, op1=ALU.mult)
        t2 = accum.tile([128, NT], F32, tag="t2")
        nc.vector.tensor_mul(t2[:], dns[:], ra[:, 0:NT])
        ngv = accum.tile([128, NT], F32, tag="ngv")
        nc.vector.scalar_tensor_tensor(ngv[:], t2[:], 10.0, ra[:, 2 * NT:3 * NT],
                                       op0=ALU.mult, op1=ALU.mult)

        ep = accum.tile([128, NT], F32, tag="ep")
        nc.scalar.activation(ep[:], pos[:], ACTF.Exp)
        en = accum.tile([128, NT], F32, tag="en")
        nc.scalar.activation(en[:], ngv[:], ACTF.Exp)
        ssum = accum.tile([128, NT], F32, tag="ssum")
        nc.vector.scalar_tensor_tensor(ssum[:], ep[:], EPS_LOSS, en[:],
                                       op0=ALU.add, op1=ALU.add)
        lg = accum.tile([128, NT], F32, tag="lg")
        nc.scalar.activation(lg[:], ssum[:], ACTF.Ln)
        li = accum.tile([128, NT], F32, tag="li")
        nc.vector.tensor_sub(li[:], lg[:], pos[:])
        lsum = accum.tile([128, 1], F32, tag="lsum")
        nc.vector.reduce_sum(lsum[:], li[:], axis=AX.X)

        tot_ps = ps_aux.tile([1, 1], F32, tag="aux")
        nc.tensor.matmul(tot_ps[:], lsum[:], ones128[:], start=True, stop=True)
        outt = mt_p.tile([1, 1], F32, tag="outt")
        nc.scalar.activation(outt[:], tot_ps[:], ACTF.Copy)
        nc.sync.dma_start(out_d[:, :], outt[:])

    nc.compile()
    return nc


def get_module():
    if "nc" not in _CACHE:
        _CACHE["nc"] = _build_module()
    return _CACHE["nc"]


def make_in_maps(q_b, k_b, q_grid, k_grid, labels, neg_noise):
    q_b = np.ascontiguousarray(np.asarray(q_b, dtype=np.float32)).reshape(B, C, N)
    k_b = np.ascontiguousarray(np.asarray(k_b, dtype=np.float32)).reshape(B, C, N)
    q_grid = np.ascontiguousarray(np.asarray(q_grid, dtype=np.float32)).reshape(B, C, N)
    k_grid = np.ascontiguousarray(np.asarray(k_grid, dtype=np.float32)).reshape(B, C, N)
    labels = np.asarray(labels)
    neg_noise = np.asarray(neg_noise, dtype=np.float32)

    # negative-sample index prep (O(B^2), matches jnp argmax tie-breaking)
    mask = labels[None, :] != labels[:, None]
    scores = np.where(mask, neg_noise, -np.inf)
    neg_idx = np.argmax(scores, axis=1)
    kng = k_grid[neg_idx]  # [B, C, N]

    mt = N // 128
    cst_ind = np.zeros((C, mt, mt), dtype=np.float32)
    cst_indT = np.zeros((mt, mt, C), dtype=np.float32)
    for j in range(mt):
        cst_ind[:, j, j] = 1.0
        cst_indT[j, j, :] = 1.0
    cst_ind = cst_ind.reshape(C, mt * mt)
    cst_indT = np.ascontiguousarray(cst_indT.transpose(1, 0, 2)).reshape(mt, mt * C)

    in_maps = []
    for ci in range(NCORES):
        sl = slice(ci * SPC, (ci + 1) * SPC)
        in_maps.append({
            "qb": np.ascontiguousarray(q_b[sl]).reshape(SPC * C, N),
            "kb": np.ascontiguousarray(k_b[sl]).reshape(SPC * C, N),
            "qgt": np.ascontiguousarray(q_grid[sl].transpose(0, 2, 1)).reshape(SPC * N, C),
            "kgt": np.ascontiguousarray(k_grid[sl].transpose(0, 2, 1)).reshape(SPC * N, C),
            "kngt": np.ascontiguousarray(kng[sl].transpose(0, 2, 1)).reshape(SPC * N, C),
            "cst_ind": cst_ind,
            "cst_indT": cst_indT,
        })
    return in_maps


def kernel(q_b, k_b, q_grid, k_grid, labels, neg_noise):
    global LAST_EXEC_TIME_NS
    in_maps = make_in_maps(q_b, k_b, q_grid, k_grid, labels, neg_noise)
    nc = get_module()
    from concourse.bass_utils import run_bass_kernel_spmd
    res = run_bass_kernel_spmd(nc, in_maps, core_ids=list(range(NCORES)))
    LAST_EXEC_TIME_NS = res.exec_time_ns
    total = sum(float(res.results[i]["out"][0, 0]) for i in range(NCORES))
    return np.float32(total / float(B * N))
